# revision 33
# baseline (speedup 1.0000x reference)
"""MultiLabelSoftMax loss kernel for 8 Trainium2 NeuronCores.

Math (per row b of predictions [B, C], with K positive labels l_bk):
    E_b    = sum_c exp(pred[b, c])
    spe_b  = sum_k exp(pred[b, l_bk])
    S_b    = E_b - spe_b                 (= sum of exp over negatives)
    loss   = sum_{b,k} [ log(exp(pred[b, l_bk]) + S_b) - pred[b, l_bk] ] / (B*K)

Sharding: batch rows split evenly across 8 cores (data parallel). Each core
streams its [512, 8192] slice once (memory-bound roofline) computing exp +
row-sum on the scalar engine; the 4096 positive logits are fetched with four
1024-index gpsimd dma_gather calls (64-element blocks, int16 wrapped indices
built on-device via a DRAM bounce) followed by an iota/compare/reduce select.
Each core writes one partial sum; the host adds the 8 partials and divides.

Row r of a core's slice lives at partition r % 128, slot i = r // 128
(i = 2h + u: half h, subrow u). Gather index j = u*1024 + k*128 + p per half
keeps the gather output partition-aligned with that layout (out block
q = j//128 = u*8 + k on partition p = j%128).
"""

import numpy as np

B, C, K = 4096, 8192, 8
NCORES = 8
RPC = B // NCORES          # rows per core = 512
P = 128                    # SBUF partitions
RT = RPC // P              # row tiles per core = 4
CCHUNK = 1024              # class columns per streamed chunk (512 KB tiles)
NCH = C // CCHUNK          # chunks per row tile = 8
BLK = 64                   # dma_gather block: 64 f32 = 256B
NH = 2                     # halves (int16 block-index range)
HIDX = RPC // NH * K       # gather indices per half = 2048

_CACHE = {}


def _build(debug_outputs=False):
    import concourse.bacc as bacc
    import concourse.bass as bass
    import concourse.tile as tile
    from concourse import mybir

    nc = bacc.Bacc("TRN2", target_bir_lowering=False, debug=False,
                   num_devices=NCORES)
    f32 = mybir.dt.float32
    i32 = mybir.dt.int32
    i16 = mybir.dt.int16

    preds = nc.declare_dram_parameter("preds", [RPC, C], f32, isOutput=False)
    labels = nc.declare_dram_parameter("labels", [RPC, K], i32, isOutput=False)
    partial = nc.declare_dram_parameter("partial", [1, 1], f32, isOutput=True)
    if debug_outputs:
        dbg_pos = nc.declare_dram_parameter("dbg_pos", [P, RT * K], f32,
                                            isOutput=True)
        dbg_e = nc.declare_dram_parameter("dbg_e", [P, RT], f32, isOutput=True)
    scratch = nc.dram_tensor("scratch", [NH * HIDX], i16)

    with tile.TileContext(nc) as tc:
        with (
            tc.tile_pool(name="big", bufs=6) as big,
            tc.tile_pool(name="small", bufs=1) as small,
            tc.tile_pool(name="psum", bufs=1, space="PSUM") as psum,
        ):
            # ---- positive-logit gather path (tiny; Pool + DVE only) ----
            labels_s = small.tile([P, RT, K], i32)
            nc.gpsimd.dma_start(
                out=labels_s[:],
                in_=labels.ap().rearrange("(i p) k -> p i k", p=P),
            )
            # Gather index j = 1024u + 128k + 16pg + pp addresses positive
            # (row r = h*256 + u*128 + pg*16 + pp, label k); the ucode wants
            # index j at partition j%16 (= pp), free slot j//16 (= 16k+8u+pg),
            # replicated across the eight 16-partition groups. Build the
            # wrapped list directly on partitions 0-15:
            #   1. load labels pre-wrapped with an int32->int16 casting DMA
            #   2. block index = (label >> 6) + u*16384 + pg*2048 + pp*128
            #   3. replicate to 128 partitions via a DRAM bounce (contiguous
            #      write + partition-broadcast read)
            wl = small.tile([16, NH, NH, 8, K], i16)       # free (h, u, pg, k)
            for h in range(NH):
                for u in range(NH):
                    nc.gpsimd.dma_start(
                        out=wl[:, h, u, :, :],
                        in_=bass.AP(tensor=labels.ap().tensor,
                                    offset=h * 2048 + u * 1024,
                                    ap=[[8, 16], [128, 8], [1, K]]),
                    )
            widx16 = small.tile([16, NH, NH, K, 8], i16)   # free (h, u, k, pg)
            a16 = small.tile([16, NH, K, 8], i16)          # free (u, k, pg)
            nc.gpsimd.iota(
                a16[:], pattern=[[16384, NH], [0, K], [2048, 8]],
                base=0, channel_multiplier=C // BLK,
            )
            nc.vector.tensor_scalar(
                out=widx16[:].rearrange("p h u k g -> p h u g k"), in0=wl[:],
                scalar1=6, scalar2=None,
                op0=mybir.AluOpType.logical_shift_right,
            )
            for h in range(NH):
                nc.vector.tensor_tensor(
                    out=widx16[:, h], in0=widx16[:, h], in1=a16[:],
                    op=mybir.AluOpType.add,
                )
            nc.gpsimd.dma_start(
                out=bass.AP(tensor=scratch, offset=0,
                            ap=[[NH * HIDX // 16, 16], [1, NH * HIDX // 16]]),
                in_=widx16[:].rearrange("p a b c d -> p (a b c d)"),
            )
            widx = small.tile([P, NH, HIDX // 16], i16)
            nc.gpsimd.dma_start(
                out=widx[:].rearrange("p a b -> p (a b)"),
                in_=bass.AP(tensor=scratch, offset=0,
                            ap=[[0, 8], [NH * HIDX // 16, 16],
                                [1, NH * HIDX // 16]]),
            )
            # >1024 indices per dma_gather deadlocks the SWDGE ring (probed:
            # 1024 ok, 2048 hangs) -> split each half into 1024-idx calls
            GCH = 1024
            gath = small.tile([P, NH, HIDX // P, BLK], f32)
            for h in range(NH):
                for c in range(HIDX // GCH):
                    nc.gpsimd.dma_gather(
                        out_ap=gath[:, h, c * (GCH // P):(c + 1) * (GCH // P), :],
                        in_ap=bass.AP(tensor=preds.ap().tensor,
                                      offset=h * (RPC // NH) * C,
                                      ap=[[BLK, RPC // NH * C // BLK], [1, BLK]]),
                        idxs_ap=widx[:, h, c * (GCH // 16):(c + 1) * (GCH // 16)],
                        num_idxs=GCH,
                        num_idxs_reg=GCH,
                        elem_size=BLK,
                    )
            # select element (label & 63) from each 64-block:
            # gath[p, h, q=u*8+k, :] -> pos_vals[p, i=2h+u, k]
            off = small.tile([P, RT, K], i32)
            nc.vector.tensor_scalar(
                out=off[:], in0=labels_s[:], scalar1=BLK - 1, scalar2=None,
                op0=mybir.AluOpType.bitwise_and,
            )
            iota64 = small.tile([P, RT, K, BLK], i32)
            nc.gpsimd.iota(
                iota64[:], pattern=[[0, RT], [0, K], [1, BLK]],
                base=0, channel_multiplier=0,
            )
            pos_vals = small.tile([P, RT, K], f32)
            sel = small.tile([P, RT, K, BLK], f32)
            # off iterated (p, i, k) with a stride-0 block dim appended
            offap = off[:]   # dims (p, i=(h,u), k): strides [part, K, 1]
            off_bc = bass.AP(
                tensor=offap.tensor, offset=offap.offset,
                ap=[offap.ap[0], offap.ap[1], offap.ap[2], [0, BLK]],
            )
            nc.vector.tensor_tensor(
                out=sel[:], in0=iota64[:], in1=off_bc,
                op=mybir.AluOpType.is_equal,
            )
            nc.vector.tensor_mul(
                out=sel[:], in0=sel[:],
                in1=gath[:].rearrange("p h (u k) d -> p (h u) k d", u=NH),
            )
            nc.vector.tensor_reduce(
                out=pos_vals[:], in_=sel[:],
                axis=mybir.AxisListType.X, op=mybir.AluOpType.add,
            )

            # posacc = sum of positive logits per partition (early, overlapped)
            posacc = small.tile([P, 1], f32)
            nc.vector.tensor_reduce(
                out=posacc[:], in_=pos_vals[:].rearrange("p a b -> p (a b)"),
                axis=mybir.AxisListType.X, op=mybir.AluOpType.add,
            )

            # ---- main stream: exp + row-sum over all classes ----
            esum = small.tile([P, RT, NCH], f32)
            last_exp = None
            for i in range(RT):
                for j in range(NCH):
                    chunk = big.tile([P, CCHUNK], f32)
                    nc.sync.dma_start(
                        out=chunk[:],
                        in_=preds[i * P:(i + 1) * P,
                                  j * CCHUNK:(j + 1) * CCHUNK],
                    )
                    last_exp = nc.scalar.activation(
                        out=chunk[:], in_=chunk[:],
                        func=mybir.ActivationFunctionType.Exp,
                        accum_out=esum[:, i, j:j + 1],
                    )

            # ---- epilogue tail (as short as possible after the last chunk) --
            e_row = small.tile([P, RT], f32)
            nc.vector.tensor_reduce(
                out=e_row[:], in_=esum[:],
                axis=mybir.AxisListType.X, op=mybir.AluOpType.add,
            )
            if debug_outputs:
                nc.sync.dma_start(out=dbg_pos.ap(),
                                  in_=pos_vals[:].rearrange("p a b -> p (a b)"))
                nc.sync.dma_start(out=dbg_e.ap(), in_=e_row[:])
            pe = small.tile([P, RT, K], f32)
            pe_exp = nc.scalar.activation(
                out=pe[:], in_=pos_vals[:],
                func=mybir.ActivationFunctionType.Exp,
            )
            # keep the gather-dependent ACT op out of the chunk-Exp stream:
            # ACT executes its queue in order, so an early pe-Exp would stall
            # every later chunk Exp on the whole gather chain
            from concourse.tile_rust import add_dep_helper
            add_dep_helper(pe_exp.ins, last_exp.ins,
                           reason="order pos-exp after the streamed chunks")
            spe = small.tile([P, RT], f32)
            nc.vector.tensor_reduce(
                out=spe[:], in_=pe[:],
                axis=mybir.AxisListType.X, op=mybir.AluOpType.add,
            )
            sneg = small.tile([P, RT], f32)
            nc.vector.tensor_sub(out=sneg[:], in0=e_row[:], in1=spe[:])
            # w = exp(pos) + S_neg, broadcast over k in one op
            sneg_bc = bass.AP(
                tensor=sneg[:].tensor, offset=sneg[:].offset,
                ap=[sneg[:].ap[0], sneg[:].ap[1], [0, K]],
            )
            nc.vector.tensor_tensor(
                out=pe[:], in0=pe[:], in1=sneg_bc, op=mybir.AluOpType.add,
            )
            # sum over all positives of log(exp(pos)+S) via the ACT accumulator
            lnacc = small.tile([P, 1], f32)
            nc.scalar.activation(
                out=pe[:].rearrange("p a b -> p (a b)"),
                in_=pe[:].rearrange("p a b -> p (a b)"),
                func=mybir.ActivationFunctionType.Ln,
                accum_out=lnacc[:],
            )
            red = small.tile([P, 1], f32)
            nc.vector.tensor_sub(out=red[:], in0=lnacc[:], in1=posacc[:])
            # partition reduction via PE: red.T @ ones -> [1, 1]
            ones = small.tile([P, 1], f32)
            nc.vector.memset(ones[:], 1.0)
            tot_ps = psum.tile([1, 1], f32)
            nc.tensor.matmul(
                out=tot_ps[:], lhsT=red[:], rhs=ones[:], start=True, stop=True,
            )
            tot = small.tile([1, 1], f32)
            nc.vector.tensor_copy(out=tot[:], in_=tot_ps[:])
            nc.sync.dma_start(out=partial.ap(), in_=tot[:])

    nc.compile()
    return nc


def kernel(predictions: np.ndarray, labels: np.ndarray) -> np.ndarray:
    from concourse.bass_utils import run_bass_kernel_spmd

    if "nc" not in _CACHE:
        _CACHE["nc"] = _build()
    nc = _CACHE["nc"]

    predictions = np.ascontiguousarray(predictions, dtype=np.float32)
    labels = np.ascontiguousarray(labels, dtype=np.int32)
    in_maps = [
        {
            "preds": predictions[c * RPC:(c + 1) * RPC],
            "labels": labels[c * RPC:(c + 1) * RPC],
        }
        for c in range(NCORES)
    ]
    res = run_bass_kernel_spmd(nc, in_maps, core_ids=list(range(NCORES)))
    total = np.float32(0.0)
    for c in range(NCORES):
        total += np.float32(res.results[c]["partial"][0, 0])
    return np.asarray(total / np.float32(B * K), dtype=np.float32)
